# revision 9
# baseline (speedup 1.0000x reference)
"""Space-to-depth (8x8 chessboard) kernel for Trainium2.

Full input  : (32, 256, 256, 32) f32
Full output : (32, 8, 8, 32768) f32
out[b, i, j] = inputs[b, i*32:(i+1)*32, j*32:(j+1)*32, :].reshape(-1)

Sharding: batch dim (32) split across 8 NeuronCores -> 4 examples/core.
Per core the whole op is pure HBM->HBM data movement: for each
(example b, row-band i) one DMA with a 3D access pattern moves 1 MiB in
4 KiB contiguous chunks. 32 independent DMAs per core.
"""

import os

import numpy as np

_B_PER_CORE = 4
_N_CORES = 8
_IN_SHAPE = (_B_PER_CORE, 256, 256, 32)
_OUT_SHAPE = (_B_PER_CORE, 8, 8, 32768)
_EX = 256 * 256 * 32      # elements per example  (2097152)
_BAND = 32 * 256 * 32     # elements per (example, row-band)  (262144)

_CACHE = {}


def _aps(bass, x, y, b, i, order, r0=0, nr=32):
    """src/dst APs for the (b, i) cell-band move, rows r0..r0+nr-1.

    order 'jr': iterate (j, r, e) -> dst contiguous, outer count 8
    order 'rj': iterate (r, j, e) -> src contiguous runs, outer count nr
    """
    off = b * _EX + i * _BAND
    if order == "jr":
        assert r0 == 0 and nr == 32
        src = bass.AP(x, off, [[1024, 8], [8192, 32], [1, 1024]])
        dst = bass.AP(y, off, [[32768, 8], [1024, 32], [1, 1024]])
    else:
        src = bass.AP(x, off + r0 * 8192, [[8192, nr], [1024, 8], [1, 1024]])
        dst = bass.AP(y, off + r0 * 1024, [[1024, nr], [32768, 8], [1, 1024]])
    return src, dst


def build_nc(variant=None):
    import concourse.bass as bass
    import concourse.mybir as mybir

    variant = variant or os.environ.get("KERNEL_VARIANT", "d")

    nc = bass.Bass(target_bir_lowering=False)
    x = nc.dram_tensor("x", list(_IN_SHAPE), mybir.dt.float32, kind="ExternalInput")
    y = nc.dram_tensor("y", list(_OUT_SHAPE), mybir.dt.float32, kind="ExternalOutput")

    def issue(engine, my_jobs, sem, order):
        n = 0
        for b, i, r0, nr in my_jobs:
            src, dst = _aps(bass, x, y, b, i, order, r0, nr)
            engine.dma_start(out=dst, in_=src).then_inc(sem, 16)
            n += 16
        if n:
            engine.wait_ge(sem, n)

    full = [(b, i, 0, 32) for b in range(_B_PER_CORE) for i in range(8)]
    halves = [
        (b, i, h * 16, 16)
        for b in range(_B_PER_CORE)
        for i in range(8)
        for h in range(2)
    ]

    # variant -> (engine splits, AP order, job list)
    # engine splits: list of engine names; jobs round-robin across them
    cfg = {
        "a": (["sync", "scalar"], "jr", full),
        "b": (["sync", "scalar"], "rj", full),
        "g": (["gpsimd"], "rj", full),
        "sg": (["sync", "scalar", "gpsimd"], "rj", full),
        "s1": (["sync"], "rj", full),
        "d": (["sync", "scalar", "gpsimd"], "rj", halves),
        "e": (["sync", "scalar", "gpsimd"], "rj", halves),
        "f": (["scalar", "sync", "gpsimd"], "rj", halves),
    }[variant]
    engines, order, jobs = cfg
    block_assign = variant == "e"

    with (
        nc.semaphore("sem0") as sem0,
        nc.semaphore("sem1") as sem1,
        nc.semaphore("sem2") as sem2,
        nc.Block(no_gpsimd_drain="gpsimd" not in engines) as block,
    ):
        sems = [sem0, sem1, sem2]
        ne = len(engines)
        for k, ename in enumerate(engines):
            if block_assign:
                lo = (len(jobs) * k) // ne
                hi = (len(jobs) * (k + 1)) // ne
                my_jobs = jobs[lo:hi]
            else:
                my_jobs = jobs[k::ne]
            sem = sems[k]

            def body(eng, my_jobs=my_jobs, sem=sem):
                issue(eng, my_jobs, sem, order)

            getattr(block, ename)(body)

    return nc


def _get_nc():
    if "nc" not in _CACHE:
        _CACHE["nc"] = build_nc()
    return _CACHE["nc"]


def kernel(inputs: np.ndarray) -> np.ndarray:
    from concourse.bass_utils import run_bass_kernel_spmd

    inputs = np.ascontiguousarray(np.asarray(inputs, dtype=np.float32))
    assert inputs.shape == (_B_PER_CORE * _N_CORES,) + _IN_SHAPE[1:]

    nc = _get_nc()
    in_maps = [
        {"x": np.ascontiguousarray(inputs[c * _B_PER_CORE : (c + 1) * _B_PER_CORE])}
        for c in range(_N_CORES)
    ]
    res = run_bass_kernel_spmd(nc, in_maps, core_ids=list(range(_N_CORES)))
    return np.concatenate([r["y"] for r in res.results], axis=0)
